# revision 7
# baseline (speedup 1.0000x reference)
"""GAT (2-layer, PyG-style) on 8 Trainium2 NeuronCores.

Strategy (edge parallelism; nodes split into 8 contiguous ranges, each core
owns all in-edges of its nodes, so no cross-core reduction is needed):

  - Launch A: R1 = x @ [W1 | W1@att_src | W1@att_dst] per node on the PE
    (each core does its 12.5K-node shard from a host-transposed x slice).
  - Host gathers a 34-col record per edge slot: [h (32) | logit (2)] where
    logit = a_src[src] + a_dst[dst] (the add is free host prep on top of the
    gather). Records are laid out channel-major per 128-dst-node tile
    ([lane=dst, ch, slot]) in a padded-CSR slot layout; per-core nodes are
    degree-sorted and slot counts quantized per 8-tile chunk so one DMA +
    one instruction covers 8 tiles.
  - Launch B: per chunk: e = Exp(Lrelu(logit)) on ACT; V = e*h in-place on
    DVE (bf16 2x mode); segment-sum over the slot axis via a pairwise
    halving tree (bf16 tensor_tensor adds at 2x, final level f32) instead
    of the 1x tensor_reduce; denominators via a small tensor_reduce on e.
    Then batched normalize + bias + ELU, and R2 = elu @ [W2 | W2@att_src2 |
    W2@att_dst2] via PE transpose (4 tiles per transpose) + matmul.
  - Pad slots carry logit = -1e6 so e == 0 exactly: pads vanish.
  - Host gathers layer-2 records [h2 (2) | logit2 (1)] from R2; Launch C
    repeats the aggregation for layer 2 + log_softmax.
"""

import sys

sys.path.insert(0, "/opt/trn_rl_repo")

from contextlib import ExitStack

import ml_dtypes
import numpy as np

import concourse.tile as tile
from concourse import bass, mybir
from concourse.bass_utils import run_bass_kernel_spmd
from concourse.masks import make_identity

F32 = mybir.dt.float32
BF16 = mybir.dt.bfloat16
BF = ml_dtypes.bfloat16

NC = 8
TILE = 128
CHUNK_T = 8  # tiles per compute chunk (shared slot-count D within a chunk)
XA_CH = 14  # tiles per DMA chunk in launch A
NEG_SLOPE = 0.2
BIG_NEG = -1.0e6


_ws_seq = [0]


def _split_waits(nc, limit=1):
    """The walrus build in this container rejects instructions carrying more
    than one sem wait ("Too many sync wait commands"). Hoist excess waits
    onto NOP carriers inserted just before the instruction (same engine, same
    program order, so semantics are preserved)."""
    for f in nc.m.functions:
        for blk in f.blocks:
            il = list(blk.instructions)
            out = []
            changed = False
            for inst in il:
                si = inst.sync_info
                waits = list(si.on_wait) if (si and si.on_wait) else []
                if len(waits) > limit:
                    keep = waits[-limit:]
                    for w in waits[:-limit]:
                        _ws_seq[0] += 1
                        nop = mybir.InstNoOp(name=f"WS-{_ws_seq[0]}")
                        nop.engine = inst.engine
                        nop.sync_info = mybir.SyncInfo(on_wait=[w], on_update=[])
                        out.append(nop)
                    si.on_wait = keep
                    changed = True
                out.append(inst)
            if changed:
                blk.instructions = out


# ---------------------------------------------------------------- host prep


def _plan(src, dst, n_nodes, n_cores):
    """Node ranges, degree-sorted tiles, chunk-quantized slot counts Dt,
    slot src ids."""
    per = n_nodes // n_cores
    ntiles = (per + TILE - 1) // TILE
    padn = ntiles * TILE

    deg = np.bincount(dst, minlength=n_nodes)

    order_e = np.argsort(dst, kind="stable")
    s_src = src[order_e]
    rowptr = np.zeros(n_nodes + 1, dtype=np.int64)
    np.cumsum(deg, out=rowptr[1:])

    orders = []  # per core: global node id per sorted slot lane (-1 = fake)
    Dt_all = np.zeros((n_cores, ntiles), dtype=np.int64)
    for c in range(n_cores):
        d = deg[c * per : (c + 1) * per]
        ids = np.concatenate(
            [c * per + np.arange(per), np.full(padn - per, -1, np.int64)]
        )
        dd = np.concatenate([d, np.zeros(padn - per, np.int64)])
        o = np.argsort(dd, kind="stable")
        orders.append(ids[o])
        Dt_all[c] = dd[o].reshape(ntiles, TILE).max(axis=1)
    Dmax = Dt_all.max(axis=0)

    # shared schedule: per 8-tile chunk, D = max over chunk, rounded to even
    Dt = np.zeros(ntiles, dtype=np.int64)
    for k0 in range(0, ntiles, CHUNK_T):
        D = int(Dmax[k0 : k0 + CHUNK_T].max())
        D = max(2, D + (D & 1))
        Dt[k0 : k0 + CHUNK_T] = D
    nblocks = int(Dt.sum())

    # slot src ids per core: [nblocks, TILE] int64, pad = n_nodes
    slot_src = np.full((n_cores, nblocks, TILE), n_nodes, dtype=np.int64)
    for c in range(n_cores):
        ids = orders[c]
        b0 = 0
        for t in range(ntiles):
            D = int(Dt[t])
            nid = ids[t * TILE : (t + 1) * TILE]
            real = nid >= 0
            nid_c = np.where(real, nid, 0)
            degs = np.where(real, deg[nid_c], 0)
            jj = np.arange(D)[:, None]  # [D, TILE]
            valid = jj < degs[None, :]
            eidx = rowptr[nid_c][None, :] + np.minimum(jj, np.maximum(degs - 1, 0))
            vals = s_src[np.clip(eidx, 0, len(s_src) - 1)]
            slot_src[c, b0 : b0 + D] = np.where(valid, vals, n_nodes)
            b0 += D
    return per, ntiles, padn, Dt, nblocks, slot_src, orders


def _records(tab, adst, slot_src_c, ids_c, Dt):
    """Channel-major per-tile records: [TILE, sum_t ncols*D_t] bf16.
    tab: [n+1, ncols] f32 (row n = pad: h=0, logit cols = -BIG).
    adst: [n, nh] f32 added onto the last nh cols per dst lane."""
    ncols = tab.shape[1]
    nh = adst.shape[1]
    ntiles = len(Dt)
    vals = tab[slot_src_c]  # [nblocks, TILE, ncols] f32
    ad = np.where((ids_c >= 0)[:, None], adst[np.maximum(ids_c, 0)], 0.0)
    adb = np.repeat(
        ad.reshape(ntiles, TILE, nh), Dt, axis=0
    )  # [nblocks, TILE, nh]
    vals[:, :, ncols - nh :] += adb
    parts = []
    b0 = 0
    for D in Dt:
        D = int(D)
        parts.append(
            vals[b0 : b0 + D].transpose(1, 2, 0).reshape(TILE, ncols * D)
        )
        b0 += D
    return np.ascontiguousarray(np.concatenate(parts, axis=1)).astype(BF)


# ---------------------------------------------- launch A (per-node matmul)


def _build_a(ntiles, padn, fdim, rec, repeat=None):
    """r1[n, :] = x[n, :] @ w1p for the core's node shard (natural order)."""
    nc = bass.Bass("TRN2")
    xtp = nc.declare_dram_parameter("xtp", [fdim, padn], BF16, isOutput=False)
    w1t_p = nc.declare_dram_parameter("w1t", [fdim, rec], BF16, isOutput=False)
    r1 = nc.declare_dram_parameter("r1", [TILE, ntiles * rec], F32, isOutput=True)

    with ExitStack() as ctx:
        tc = ctx.enter_context(tile.TileContext(nc))
        const = ctx.enter_context(tc.tile_pool(name="const", bufs=1))
        xpool = ctx.enter_context(tc.tile_pool(name="xa", bufs=3))
        rspool = ctx.enter_context(tc.tile_pool(name="rs", bufs=3))
        pspool = ctx.enter_context(tc.tile_pool(name="ps", bufs=8, space="PSUM"))

        w1t = const.tile([fdim, rec], BF16)
        nc.sync.dma_start(out=w1t[:], in_=w1t_p[:])

        if repeat:
            ctx.enter_context(tc.For_i(0, repeat, 1))
        for k0 in range(0, ntiles, XA_CH):
            T = min(XA_CH, ntiles - k0)
            xt = xpool.tile([fdim, XA_CH * TILE], BF16, tag="xa")
            nc.sync.dma_start(
                out=xt[:, 0 : T * TILE],
                in_=xtp[:, k0 * TILE : (k0 + T) * TILE],
            )
            r1s = rspool.tile([TILE, XA_CH, rec], F32, tag="r1s")
            for j in range(T):
                ps = pspool.tile([TILE, rec], F32, tag="ps")
                nc.tensor.matmul(
                    out=ps[:],
                    lhsT=xt[:, j * TILE : (j + 1) * TILE],
                    rhs=w1t[:],
                    start=True,
                    stop=True,
                )
                nc.scalar.copy(out=r1s[:, j, :], in_=ps[:])
            nc.sync.dma_start(
                out=r1[:, k0 * rec : (k0 + T) * rec],
                in_=r1s[:, 0:T, :].rearrange("n t c -> n (t c)"),
            )
    return nc


# ------------------------------------------- launch B (layer-1 aggregation)


def _tree_sum(nc, xt, T, ncols, D, acc_out):
    """Pairwise-halving sum over the slot axis of xt[:, :, 0:ncols, 0:D]
    (in-place, bf16 2x adds, even offsets), final level f32 into acc_out
    (an AP of shape [TILE, T, ncols])."""
    w = D
    while w > 2:
        nw = (w + 1) // 2
        nw += nw & 1  # keep in1 offset even for the 2x packed mode
        h2 = w - nw
        nc.vector.tensor_tensor(
            out=xt[:, :, 0:ncols, 0:h2],
            in0=xt[:, :, 0:ncols, 0:h2],
            in1=xt[:, :, 0:ncols, nw : nw + h2],
            op=mybir.AluOpType.add,
        )
        w = nw
    if w == 2:
        nc.vector.tensor_tensor(
            out=acc_out.unsqueeze(-1),
            in0=xt[:, :, 0:ncols, 0:1],
            in1=xt[:, :, 0:ncols, 1:2],
            op=mybir.AluOpType.add,
        )
    else:  # w == 1
        nc.vector.tensor_copy(
            out=acc_out.unsqueeze(-1), in_=xt[:, :, 0:ncols, 0:1]
        )


def _build_b(ntiles, Dt, padn, repeat=None):
    """Layer-1 edge aggregation from 34-col records + ELU + R2 table."""
    d1, nh, chn = 32, 2, 16
    ncols = d1 + nh  # 34
    F1 = int(ncols * Dt.sum())
    nc = bass.Bass("TRN2")
    xr = nc.declare_dram_parameter("xr", [TILE, F1], BF16, isOutput=False)
    b1r = nc.declare_dram_parameter("b1r", [TILE, d1], F32, isOutput=False)
    w2p = nc.declare_dram_parameter("w2p", [TILE, 4], F32, isOutput=False)
    r2 = nc.declare_dram_parameter("r2", [TILE, ntiles * 4], F32, isOutput=True)

    with ExitStack() as ctx:
        tc = ctx.enter_context(tile.TileContext(nc))
        const = ctx.enter_context(tc.tile_pool(name="const", bufs=1))
        xe = ctx.enter_context(tc.tile_pool(name="xe", bufs=3))
        epool = ctx.enter_context(tc.tile_pool(name="ep", bufs=3))
        vpool = ctx.enter_context(tc.tile_pool(name="vp", bufs=2))
        work = ctx.enter_context(tc.tile_pool(name="wk", bufs=2))
        outp = ctx.enter_context(tc.tile_pool(name="op", bufs=2))
        ppool = ctx.enter_context(tc.tile_pool(name="pp", bufs=2, space="PSUM"))
        rpool = ctx.enter_context(tc.tile_pool(name="rp", bufs=2, space="PSUM"))

        b1t = const.tile([TILE, d1], F32)
        nc.sync.dma_start(out=b1t[:], in_=b1r[:])
        w2t = const.tile([TILE, 4], F32)
        nc.sync.dma_start(out=w2t[:], in_=w2p[:])
        ident = const.tile([TILE, TILE], F32)
        make_identity(nc, ident[:])

        if repeat:
            ctx.enter_context(tc.For_i(0, repeat, 1))
        accn = vpool.tile([TILE, ntiles, d1], F32, tag="accn")
        accd = vpool.tile([TILE, ntiles, nh], F32, tag="accd")
        off = 0
        for t0 in range(0, ntiles, CHUNK_T):
            T = min(CHUNK_T, ntiles - t0)
            D = int(Dt[t0])
            sz = T * ncols * D
            xt = xe.tile([TILE, CHUNK_T, ncols, D], BF16, tag="xt")
            nc.sync.dma_start(
                out=xt[:, 0:T, :, :].rearrange("p t c d -> p (t c d)"),
                in_=xr[:, off : off + sz],
            )
            xtv = xt[:, 0:T, :, :]
            lr = epool.tile([TILE, CHUNK_T, nh, D], BF16, tag="lr")
            nc.scalar.activation(
                out=lr[:, 0:T, :, :],
                in_=xtv[:, :, d1 : d1 + nh, :],
                func=mybir.ActivationFunctionType.Lrelu,
                alpha=NEG_SLOPE,
            )
            et = epool.tile([TILE, CHUNK_T, nh, D], BF16, tag="et")
            nc.scalar.activation(
                out=et[:, 0:T, :, :],
                in_=lr[:, 0:T, :, :],
                func=mybir.ActivationFunctionType.Exp,
            )
            etv = et[:, 0:T, :, :]
            nc.vector.tensor_reduce(
                out=accd[:, t0 : t0 + T, :],
                in_=etv,
                axis=mybir.AxisListType.X,
                op=mybir.AluOpType.add,
            )
            for h in range(nh):
                nc.vector.tensor_tensor(
                    out=xtv[:, :, h * chn : (h + 1) * chn, :],
                    in0=xtv[:, :, h * chn : (h + 1) * chn, :],
                    in1=etv[:, :, h, :].unsqueeze(2).to_broadcast(
                        [TILE, T, chn, D]
                    ),
                    op=mybir.AluOpType.mult,
                )
            _tree_sum(nc, xtv, T, d1, D, accn[:, t0 : t0 + T, :])
            off += sz

        # ---- batched finishing across all tiles ----
        inv = work.tile([TILE, ntiles, nh], F32, tag="inv")
        nc.vector.tensor_scalar_add(out=inv[:], in0=accd[:], scalar1=1e-16)
        nc.vector.reciprocal(out=inv[:], in_=inv[:])
        o1a = vpool.tile([TILE, ntiles, d1], F32, tag="o1a")
        nc.vector.tensor_tensor(
            out=o1a[:].rearrange("p t (h c) -> p t h c", h=nh),
            in0=accn[:].rearrange("p t (h c) -> p t h c", h=nh),
            in1=inv[:].unsqueeze(-1).to_broadcast([TILE, ntiles, nh, chn]),
            op=mybir.AluOpType.mult,
        )
        nc.vector.tensor_tensor(
            out=o1a[:],
            in0=o1a[:],
            in1=b1t[:].unsqueeze(1).to_broadcast([TILE, ntiles, d1]),
            op=mybir.AluOpType.add,
        )
        # elu = max(x,0) + exp(min(x,0)) - 1
        e1 = vpool.tile([TILE, ntiles, d1], F32, tag="e1")
        nc.vector.tensor_scalar_min(out=e1[:], in0=o1a[:], scalar1=0.0)
        nc.scalar.activation(
            out=e1[:], in_=e1[:], func=mybir.ActivationFunctionType.Exp
        )
        nc.vector.tensor_scalar_add(out=e1[:], in0=e1[:], scalar1=-1.0)
        nc.vector.tensor_scalar_max(out=o1a[:], in0=o1a[:], scalar1=0.0)
        nc.vector.tensor_tensor(
            out=o1a[:], in0=o1a[:], in1=e1[:], op=mybir.AluOpType.add
        )
        # R2 = [h2 | a_src2 | a_dst2] = elu_out @ w2p, 3 tiles per transpose
        # (PE base partition must be 0/32/64)
        r2all = outp.tile([TILE, ntiles, 4], F32, tag="r2all")
        for t0 in range(0, ntiles, 3):
            tg = min(3, ntiles - t0)
            pt = ppool.tile([TILE, TILE], F32, tag="pt")
            nc.tensor.transpose(
                out=pt[0 : tg * d1, :],
                in_=o1a[:, t0 : t0 + tg, :].rearrange("p t c -> p (t c)"),
                identity=ident[:],
            )
            o1t = work.tile([TILE, TILE], F32, tag="o1t")
            nc.scalar.copy(out=o1t[0 : tg * d1, :], in_=pt[0 : tg * d1, :])
            rp = rpool.tile([TILE, 16], F32, tag="rp")
            for j in range(tg):
                nc.tensor.matmul(
                    out=rp[:, 4 * j : 4 * j + 4],
                    lhsT=o1t[j * d1 : (j + 1) * d1, :],
                    rhs=w2t[j * d1 : (j + 1) * d1, :],
                    start=True,
                    stop=True,
                )
            nc.scalar.copy(
                out=r2all[:, t0 : t0 + tg, :].rearrange("p t c -> p (t c)"),
                in_=rp[:, 0 : 4 * tg],
            )
        nc.sync.dma_start(
            out=r2[:], in_=r2all[:].rearrange("n t c -> n (t c)")
        )
    return nc


# ------------------------------------------- launch C (layer-2 aggregation)


def _build_c(ntiles, Dt, padn, repeat=None):
    """Layer 2 (1 head, 2 ch) from 3-col records + bias + log_softmax."""
    d2 = 2
    ncols = 3
    F2 = int(ncols * Dt.sum())
    nc = bass.Bass("TRN2")
    xr2 = nc.declare_dram_parameter("xr2", [TILE, F2], BF16, isOutput=False)
    b2r = nc.declare_dram_parameter("b2r", [TILE, d2], F32, isOutput=False)
    y = nc.declare_dram_parameter("y", [TILE, ntiles * d2], F32, isOutput=True)

    with ExitStack() as ctx:
        tc = ctx.enter_context(tile.TileContext(nc))
        const = ctx.enter_context(tc.tile_pool(name="const", bufs=1))
        xe = ctx.enter_context(tc.tile_pool(name="xe", bufs=3))
        epool = ctx.enter_context(tc.tile_pool(name="ep", bufs=3))
        vpool = ctx.enter_context(tc.tile_pool(name="vp", bufs=2))
        work = ctx.enter_context(tc.tile_pool(name="wk", bufs=2))
        outp = ctx.enter_context(tc.tile_pool(name="op", bufs=2))

        b2t = const.tile([TILE, d2], F32)
        nc.sync.dma_start(out=b2t[:], in_=b2r[:])

        if repeat:
            ctx.enter_context(tc.For_i(0, repeat, 1))
        accn = vpool.tile([TILE, ntiles, d2], F32, tag="accn")
        accd = vpool.tile([TILE, ntiles], F32, tag="accd")
        off = 0
        for t0 in range(0, ntiles, CHUNK_T):
            T = min(CHUNK_T, ntiles - t0)
            D = int(Dt[t0])
            sz = T * ncols * D
            xt = xe.tile([TILE, CHUNK_T, ncols, D], BF16, tag="xt")
            nc.sync.dma_start(
                out=xt[:, 0:T, :, :].rearrange("p t c d -> p (t c d)"),
                in_=xr2[:, off : off + sz],
            )
            xtv = xt[:, 0:T, :, :]
            lr = epool.tile([TILE, CHUNK_T, 1, D], BF16, tag="lr")
            nc.scalar.activation(
                out=lr[:, 0:T, :, :],
                in_=xtv[:, :, d2 : d2 + 1, :],
                func=mybir.ActivationFunctionType.Lrelu,
                alpha=NEG_SLOPE,
            )
            et = epool.tile([TILE, CHUNK_T, 1, D], BF16, tag="et")
            nc.scalar.activation(
                out=et[:, 0:T, :, :],
                in_=lr[:, 0:T, :, :],
                func=mybir.ActivationFunctionType.Exp,
            )
            etv = et[:, 0:T, :, :]
            nc.vector.tensor_reduce(
                out=accd[:, t0 : t0 + T].unsqueeze(-1),
                in_=etv,
                axis=mybir.AxisListType.X,
                op=mybir.AluOpType.add,
            )
            nc.vector.tensor_tensor(
                out=xtv[:, :, 0:d2, :],
                in0=xtv[:, :, 0:d2, :],
                in1=etv[:, :, 0, :].unsqueeze(2).to_broadcast([TILE, T, d2, D]),
                op=mybir.AluOpType.mult,
            )
            _tree_sum(nc, xtv, T, d2, D, accn[:, t0 : t0 + T, :])
            off += sz

        # ---- batched finishing ----
        inv = work.tile([TILE, ntiles], F32, tag="inv")
        nc.vector.tensor_scalar_add(out=inv[:], in0=accd[:], scalar1=1e-16)
        nc.vector.reciprocal(out=inv[:], in_=inv[:])
        z = vpool.tile([TILE, ntiles, d2], F32, tag="z")
        nc.vector.tensor_tensor(
            out=z[:],
            in0=accn[:],
            in1=inv[:].unsqueeze(-1).to_broadcast([TILE, ntiles, d2]),
            op=mybir.AluOpType.mult,
        )
        nc.vector.tensor_tensor(
            out=z[:],
            in0=z[:],
            in1=b2t[:].unsqueeze(1).to_broadcast([TILE, ntiles, d2]),
            op=mybir.AluOpType.add,
        )
        # log_softmax over the 2 columns
        m = work.tile([TILE, ntiles], F32, tag="m")
        nc.vector.tensor_reduce(
            out=m[:], in_=z[:], axis=mybir.AxisListType.X, op=mybir.AluOpType.max
        )
        nc.vector.tensor_tensor(
            out=z[:],
            in0=z[:],
            in1=m[:].unsqueeze(-1).to_broadcast([TILE, ntiles, d2]),
            op=mybir.AluOpType.subtract,
        )
        ez = vpool.tile([TILE, ntiles, d2], F32, tag="ez")
        nc.scalar.activation(
            out=ez[:], in_=z[:], func=mybir.ActivationFunctionType.Exp
        )
        ss = work.tile([TILE, ntiles], F32, tag="ss")
        nc.vector.tensor_reduce(
            out=ss[:],
            in_=ez[:],
            axis=mybir.AxisListType.X,
            op=mybir.AluOpType.add,
        )
        nc.scalar.activation(
            out=ss[:], in_=ss[:], func=mybir.ActivationFunctionType.Ln
        )
        yt = outp.tile([TILE, ntiles, d2], F32, tag="yt")
        nc.vector.tensor_tensor(
            out=yt[:],
            in0=z[:],
            in1=ss[:].unsqueeze(-1).to_broadcast([TILE, ntiles, d2]),
            op=mybir.AluOpType.subtract,
        )
        nc.sync.dma_start(
            out=y[:], in_=yt[:].rearrange("n t c -> n (t c)")
        )
    return nc


# ------------------------------------------------------------------- driver


def _run_gat(x, edge_index, W1, att_src1, att_dst1, b1, W2, att_src2, att_dst2, b2,
             n_cores=NC, timing=None):
    import time as _time

    n_nodes, fdim = x.shape
    nh, chn = att_src1.shape  # 2, 16
    d1 = nh * chn  # 32
    rec = d1 + 2 * nh  # h | a_src | a_dst = 36

    src = np.concatenate([np.asarray(edge_index[0]), np.arange(n_nodes)]).astype(
        np.int64
    )
    dst = np.concatenate([np.asarray(edge_index[1]), np.arange(n_nodes)]).astype(
        np.int64
    )

    per, ntiles, padn, Dt, nblocks, slot_src, orders = _plan(
        src, dst, n_nodes, n_cores
    )

    W1 = np.asarray(W1, np.float32)
    att_src1 = np.asarray(att_src1, np.float32)
    att_dst1 = np.asarray(att_dst1, np.float32)
    W2 = np.asarray(W2, np.float32)
    att_src2 = np.asarray(att_src2, np.float32)
    att_dst2 = np.asarray(att_dst2, np.float32)

    # fused weights
    w_asrc1 = np.stack(
        [W1[:, h * chn : (h + 1) * chn] @ att_src1[h] for h in range(nh)], axis=1
    )  # [F, nh]
    w_adst1 = np.stack(
        [W1[:, h * chn : (h + 1) * chn] @ att_dst1[h] for h in range(nh)], axis=1
    )
    w1p = np.concatenate([W1, w_asrc1, w_adst1], axis=1)  # [F, 36]
    w_asrc2 = W2 @ att_src2[0]
    w_adst2 = W2 @ att_dst2[0]
    w2p = np.concatenate(
        [W2, w_asrc2[:, None], w_adst2[:, None]], axis=1
    ).astype(np.float32)  # [32, 4]
    w2p = np.tile(w2p, (4, 1))  # replicated per 32-partition block for PE

    # ---- launch A: per-node R1 = x @ w1p ----
    xT = np.ascontiguousarray(np.asarray(x, np.float32).T.astype(BF))  # [F, n]
    w1p_bf = w1p.astype(BF)
    in_maps_a = []
    for c in range(n_cores):
        xc = np.zeros((fdim, padn), BF)
        xc[:, :per] = xT[:, c * per : (c + 1) * per]
        in_maps_a.append({"xtp": xc, "w1t": w1p_bf})
    nc_a = _build_a(ntiles, padn, fdim, rec)
    _split_waits(nc_a)
    t0 = _time.perf_counter()
    res_a = run_bass_kernel_spmd(nc_a, in_maps_a, list(range(n_cores)))
    t1 = _time.perf_counter()
    if timing is not None:
        timing["a_first_s"] = t1 - t0
        timing["in_maps_a"] = in_maps_a

    R1 = np.concatenate(
        [
            res_a.results[c]["r1"]
            .reshape(TILE, ntiles, rec)
            .transpose(1, 0, 2)
            .reshape(padn, rec)[:per]
            for c in range(n_cores)
        ],
        axis=0,
    )  # [n, 36] f32

    # ---- host gather of layer-1 records ----
    tab1 = np.zeros((n_nodes + 1, d1 + nh), np.float32)
    tab1[:n_nodes, 0:d1] = R1[:, 0:d1]
    tab1[:n_nodes, d1 : d1 + nh] = R1[:, d1 : d1 + nh]  # a_src
    tab1[n_nodes, d1 : d1 + nh] = BIG_NEG
    adst1 = R1[:, d1 + nh : d1 + 2 * nh]  # [n, nh]
    b1r = np.broadcast_to(np.asarray(b1, np.float32), (TILE, d1)).copy()
    in_maps_b = []
    for c in range(n_cores):
        xr = _records(tab1, adst1, slot_src[c], orders[c], Dt)
        in_maps_b.append({"xr": xr, "b1r": b1r, "w2p": w2p})

    nc_b = _build_b(ntiles, Dt, padn)
    _split_waits(nc_b)
    t2 = _time.perf_counter()
    res_b = run_bass_kernel_spmd(nc_b, in_maps_b, list(range(n_cores)))
    t3 = _time.perf_counter()
    if timing is not None:
        timing["b_first_s"] = t3 - t2
        timing["in_maps_b"] = in_maps_b

    # ---- host gather of layer-2 records ----
    tab2 = np.zeros((n_nodes + 1, 3), np.float32)
    adst2 = np.zeros((n_nodes, 1), np.float32)
    for c in range(n_cores):
        r2c = (
            res_b.results[c]["r2"]
            .reshape(TILE, ntiles, 4)
            .transpose(1, 0, 2)
            .reshape(padn, 4)
        )
        ids = orders[c]
        real = ids >= 0
        tab2[ids[real]] = r2c[real][:, 0:3]
        adst2[ids[real], 0] = r2c[real][:, 3]
    tab2[n_nodes, 2] = BIG_NEG
    b2r = np.broadcast_to(np.asarray(b2, np.float32), (TILE, 2)).copy()
    in_maps_c = []
    for c in range(n_cores):
        xr2 = _records(tab2, adst2, slot_src[c], orders[c], Dt)
        in_maps_c.append({"xr2": xr2, "b2r": b2r})

    nc_c = _build_c(ntiles, Dt, padn)
    _split_waits(nc_c)
    t4 = _time.perf_counter()
    res_c = run_bass_kernel_spmd(nc_c, in_maps_c, list(range(n_cores)))
    t5 = _time.perf_counter()
    if timing is not None:
        timing["c_first_s"] = t5 - t4
        timing["in_maps_c"] = in_maps_c

    out = np.zeros((n_nodes, 2), np.float32)
    for c in range(n_cores):
        yc = (
            res_c.results[c]["y"]
            .reshape(TILE, ntiles, 2)
            .transpose(1, 0, 2)
            .reshape(padn, 2)
        )
        ids = orders[c]
        real = ids >= 0
        out[ids[real]] = yc[real]
    return out


def kernel(x, edge_index, W1, att_src1, att_dst1, b1, W2, att_src2, att_dst2, b2):
    return _run_gat(
        np.asarray(x, np.float32),
        np.asarray(edge_index),
        W1,
        att_src1,
        att_dst1,
        b1,
        W2,
        att_src2,
        att_dst2,
        b2,
    )
